# Initial kernel scaffold
#
"""Multi-head self-attention TRN2 Bass kernel.

Problem: B=4, S=2048, EMB=1024, H=16 heads, dqk=dv=64.
Sharding: 8 cores; core c handles batch b=c//2 and head group g=c%2
(8 heads each). Each core computes its partial output projection
(rows of wo for its heads); host sums the two partials per batch and
adds bo.

Per-core inputs (bf16 except biases): xq_T/xkv_T [1024, 2048] (x.T),
wq/wk/wv [1024, 512], bq/bk/bv [512] f32, wo_g [512, 1024].
Output: partial out [2048, 1024] f32.

Dataflow:
  - Q/K projections j-major: QHT/KHT [512 j, 2048 s] bf16.
  - V projection t-major: VH [2048 t, 512 j] -> tiles [128, 8*66] with
    per-head blocks [64 V | 1 ones | 1 pad]; the ones column makes the
    AV matmul emit softmax denominators as row 64 of its output.
  - scores (t-major, K=64 row-tiled): S^T [128 t, 1024 s] PSUM, exp on
    ACT (scale=1/8 folded; max-free softmax, |scores| <~ 3) -> P^T bf16.
  - AV: accumulate Z~T [66, 512] over 16 t-tiles in PSUM.
  - normalize: reciprocal of D row, DMA round-trip broadcast across
    partitions, multiply + bv bias -> ZnormT [512 j, 2048 s] bf16.
  - out projection bf16 -> partial OUT [2048, 1024] f32 -> DRAM.
"""

import ml_dtypes
import numpy as np

import concourse.bass as bass
import concourse.tile as tile
from concourse import bacc, mybir
from concourse.bass_utils import run_bass_kernel_spmd

B, S, EMB, H, DH = 4, 2048, 1024, 16, 64
N_CORES = 8
HPC = H // 2          # heads per core
JC = HPC * DH         # 512: per-core projected width
VB = DH + 2           # 66: per-head V block (64 V cols + ones + pad)

F32 = mybir.dt.float32
BF16 = mybir.dt.bfloat16


def build_kernel(reps=1, mode='full'):
    nc = bacc.Bacc(
        "TRN2", target_bir_lowering=False, debug=False, num_devices=N_CORES
    )

    xq = nc.dram_tensor("xq", [EMB, S], BF16, kind="ExternalInput").ap()
    xkv = nc.dram_tensor("xkv", [EMB, S], BF16, kind="ExternalInput").ap()
    wq_d = nc.dram_tensor("wq", [EMB, JC], BF16, kind="ExternalInput").ap()
    wk_d = nc.dram_tensor("wk", [EMB, JC], BF16, kind="ExternalInput").ap()
    wv_d = nc.dram_tensor("wv", [EMB, JC], BF16, kind="ExternalInput").ap()
    bq_d = nc.dram_tensor("bq", [JC], F32, kind="ExternalInput").ap()
    bk_d = nc.dram_tensor("bk", [JC], F32, kind="ExternalInput").ap()
    bv_d = nc.dram_tensor("bv", [JC], F32, kind="ExternalInput").ap()
    wo_d = nc.dram_tensor("wo", [JC, EMB], BF16, kind="ExternalInput").ap()
    out_d = nc.dram_tensor("out", [S, EMB], F32, kind="ExternalOutput").ap()
    dr_d = nc.dram_tensor("dr_scratch", [2, S], F32).ap()  # Drecip bounce

    import contextlib

    with tile.TileContext(nc) as tc:
        with (
            tc.For_i(0, reps, 1) if reps > 1 else contextlib.nullcontext(),
            tc.tile_pool(name="persist", bufs=1) as pp,
        ):
            # persistent SBUF tensors
            qht = [pp.tile([128, S], BF16, name=f"qht{i}") for i in range(4)]
            kht = [pp.tile([128, S], BF16, name=f"kht{i}") for i in range(4)]
            vh = [pp.tile([128, HPC * VB], BF16, name=f"vh{t}")
                  for t in range(16)]
            znorm = [pp.tile([128, S], BF16, name=f"zn{i}") for i in range(4)]
            wos = [pp.tile([128, EMB], BF16, name=f"wo{j}") for j in range(4)]
            bias_q = pp.tile([128, 4], F32, name="bias_q")
            bias_k = pp.tile([128, 4], F32, name="bias_k")
            bias_v = pp.tile([64, HPC], F32, name="bias_v")  # [d, head]

            nc.sync.dma_start(bias_q[:], bq_d.rearrange("(c p) -> p c", p=128))
            nc.sync.dma_start(bias_k[:], bk_d.rearrange("(c p) -> p c", p=128))
            nc.sync.dma_start(bias_v[:], bv_d.rearrange("(h d) -> d h", d=DH))
            for j in range(4):
                nc.sync.dma_start(wos[j][:], wo_d[j * 128:(j + 1) * 128, :])
            # ones columns in vh blocks (col 64 of each 66-block); pad col 0
            for t in range(16):
                blocks = vh[t][:].rearrange("p (h c) -> p h c", c=VB)
                nc.vector.memset(blocks[:, :, DH:DH + 1], 1.0)
                nc.vector.memset(blocks[:, :, DH + 1:], 0.0)

            # ------------- fused projections + attention -------------
            # x is fully resident; Q/K projections are emitted per head
            # pair (jc == pair) and interleaved with the previous pair's
            # attention so ACT (exp) starts ~75us earlier and proj
            # matmuls fill PE gaps during exp-bound stretches. Proj
            # PSUM shares the score-tile slots (tags sp0/sp1).
            with (
                tc.tile_pool(name="sps", bufs=1, space="PSUM") as sp_pool,
                tc.tile_pool(name="avps", bufs=1, space="PSUM") as av_pool,
                tc.tile_pool(name="dreb_p", bufs=2) as dre_pool,
                tc.tile_pool(name="znsc", bufs=2) as zns_pool,
                tc.tile_pool(name="drec_p", bufs=2) as drec_pool,
            ):
                with (
                    tc.tile_pool(name="xr_p", bufs=1) as xr_pool,
                    tc.tile_pool(name="wqkv", bufs=1) as w_pool,
                ):
                    wqs = [w_pool.tile([128, JC], BF16, name=f"wqs{e}")
                           for e in range(8)]
                    wks = [w_pool.tile([128, JC], BF16, name=f"wks{e}")
                           for e in range(8)]
                    wvs = [w_pool.tile([128, JC], BF16, name=f"wvs{e}")
                           for e in range(8)]
                    xqr = [xr_pool.tile([128, S], BF16, name=f"xqr{e}")
                           for e in range(8)]
                    xkr = [xr_pool.tile([128, S], BF16, name=f"xkr{e}")
                           for e in range(8)]
                    # DMA in first-use order: Q-proj starts as soon as
                    # wq + xq land instead of after the full input burst.
                    for e in range(8):
                        nc.sync.dma_start(wqs[e][:], wq_d[e * 128:(e + 1) * 128, :])
                    for e in range(8):
                        nc.sync.dma_start(xqr[e][:], xq[e * 128:(e + 1) * 128, :])
                    for e in range(8):
                        nc.sync.dma_start(wks[e][:], wk_d[e * 128:(e + 1) * 128, :])
                    for e in range(8):
                        nc.sync.dma_start(xkr[e][:], xkv[e * 128:(e + 1) * 128, :])
                    for e in range(8):
                        nc.sync.dma_start(wvs[e][:], wv_d[e * 128:(e + 1) * 128, :])

                    def emit_proj(pair):
                        jsl = slice(pair * 128, (pair + 1) * 128)
                        for dst, ws, xr, bias in (
                            (qht, wqs, xqr, bias_q),
                            (kht, wks, xkr, bias_k),
                        ):
                            for sc in range(4):
                                ssl = slice(sc * 512, (sc + 1) * 512)
                                ps = sp_pool.tile([128, 512], F32,
                                                  tag=f"sp{sc % 2}",
                                                  name=f"sp{sc % 2}")
                                for e in range(8) if mode != "noproj" else []:
                                    nc.tensor.matmul(
                                        ps[:], ws[e][:, jsl], xr[e][:, ssl],
                                        start=(e == 0), stop=(e == 7),
                                    )
                                nc.vector.tensor_scalar_add(
                                    dst[pair][:, ssl], ps[:],
                                    bias[:, pair:pair + 1]
                                )
                        if pair == 0:
                            for tch in range(16):
                                tsl = slice(tch * 128, (tch + 1) * 128)
                                ps = sp_pool.tile([128, 512], F32,
                                                  tag=f"sp{tch % 2}",
                                                  name=f"sp{tch % 2}")
                                for e in range(8) if mode != "noproj" else []:
                                    nc.tensor.matmul(
                                        ps[:], xkr[e][:, tsl], wvs[e][:],
                                        start=(e == 0), stop=(e == 7),
                                    )
                                nc.vector.tensor_copy(
                                    vh[tch][:].rearrange(
                                        "p (h c) -> p h c", c=VB)[:, :, 0:DH],
                                    ps[:].rearrange("p (h d) -> p h d", d=DH),
                                )

                    for pair in range(4):
                        emit_proj(pair)

                def emit_attn(pair, pt_pool):
                    for s_half in range(2):
                        s0 = s_half * 1024
                        avs = {}
                        for par in range(2):
                            for i in range(2):
                                avs[(par, i)] = av_pool.tile(
                                    [VB, 512], F32, tag=f"av{par}{i}",
                                    name=f"av{par}{i}")

                        def emit_scores(t, par):
                            off = par * 64
                            lhs_s = kht[pair][off:off + 64,
                                              t * 128:(t + 1) * 128]
                            sp = sp_pool.tile([128, 1024], F32,
                                              tag=f"sp{par}",
                                              name=f"sp{par}")
                            for i in range(2) if mode != "noscores" else []:
                                nc.tensor.matmul(
                                    sp[:, i * 512:(i + 1) * 512],
                                    lhs_s,
                                    qht[pair][off:off + 64,
                                              s0 + i * 512:
                                              s0 + (i + 1) * 512],
                                    start=True,
                                    stop=True,
                                )
                            ptt = pt_pool.tile([128, 1024], BF16,
                                               tag=f"ptt{par}",
                                               name=f"ptt{par}")
                            if mode == "smallexp":
                                nc.scalar.activation(
                                    ptt[:, 0:64], sp[:, 0:64],
                                    mybir.ActivationFunctionType.Exp,
                                    scale=0.125,
                                )
                            else:
                                nc.scalar.activation(
                                    ptt[:], sp[:],
                                    mybir.ActivationFunctionType.Exp,
                                    scale=0.125,
                                )
                            return ptt

                        def emit_av(t, par, ptt):
                            h = pair * 2 + par
                            for i in range(2) if mode != "noav" else []:
                                nc.tensor.matmul(
                                    avs[(par, i)][:],
                                    vh[t][:, h * VB:(h + 1) * VB],
                                    ptt[:, i * 512:(i + 1) * 512],
                                    start=(t == 0),
                                    stop=(t == 15),
                                    skip_group_check=True,
                                )

                        prev = None
                        for t in range(16):
                            p0 = emit_scores(t, 0)
                            p1 = emit_scores(t, 1)
                            if prev is not None:
                                emit_av(prev[0], 0, prev[1])
                                emit_av(prev[0], 1, prev[2])
                            prev = (t, p0, p1)
                        emit_av(prev[0], 0, prev[1])
                        emit_av(prev[0], 1, prev[2])

                        if mode == "notail":
                            continue
                        for par in range(2):
                            h = pair * 2 + par
                            off = par * 64
                            dreb = dre_pool.tile(
                                [64, 1024], F32, tag=f"dreb{par}",
                                name=f"dreb{par}")
                            for i in range(2):
                                drc = drec_pool.tile(
                                    [66, 512], F32, tag=f"drc{par}",
                                    name=f"drc{par}")
                                nc.vector.reciprocal(
                                    drc[64:65, :],
                                    avs[(par, i)][DH:DH + 1, :],
                                )
                                nc.sync.dma_start(
                                    dr_d[par,
                                         s0 + i * 512:s0 + (i + 1) * 512],
                                    drc[64:65, :],
                                )
                            nc.sync.dma_start(
                                dreb[:],
                                dr_d[par:par + 1, s0:s0 + 1024]
                                .broadcast_to([64, 1024]),
                            )
                            zn_s = zns_pool.tile(
                                [64, 1024], BF16, tag=f"zn_s{par}",
                                name=f"zn_s{par}")
                            for i in range(2):
                                nc.vector.tensor_mul(
                                    zn_s[:, i * 512:(i + 1) * 512],
                                    avs[(par, i)][0:DH, :],
                                    dreb[:, i * 512:(i + 1) * 512],
                                )
                            nc.vector.tensor_scalar_add(
                                zn_s[:], zn_s[:], bias_v[:, h:h + 1]
                            )
                            nc.sync.dma_start(
                                znorm[pair][off:off + 64, s0:s0 + 1024],
                                zn_s[:],
                            )

                with tc.tile_pool(name="pt", bufs=10) as pt_pool:
                    for pair in range(4):
                        emit_attn(pair, pt_pool)

            # ---------------- output projection ----------------
            with (
                tc.tile_pool(name="ops", bufs=4, space="PSUM") as op_pool,
                tc.tile_pool(name="ostg", bufs=4) as ostg_pool,
            ):
                for scc in range(16):
                    psl = slice(scc * 128, (scc + 1) * 128)
                    for oc in range(2):
                        ps = op_pool.tile([128, 512], F32, tag="ops",
                                          name="ops")
                        osl = slice(oc * 512, (oc + 1) * 512)
                        for jt in range(4) if mode != "nooutproj" else []:
                            nc.tensor.matmul(
                                ps[:],
                                znorm[jt][:, psl],
                                wos[jt][:, osl],
                                start=(jt == 0),
                                stop=(jt == 3),
                            )
                        ostg = ostg_pool.tile([128, 512], F32, tag="ostg",
                                              name="ostg")
                        nc.vector.tensor_copy(ostg[:], ps[:])
                        nc.sync.dma_start(out_d[psl, osl], ostg[:])

    nc.compile()
    return nc


def _bf16(a):
    return np.asarray(a, np.float32).astype(ml_dtypes.bfloat16)


def _prep_inputs(q, k_and_v, wq, bq, wk, bk, wv, bv, wo):
    """Build per-core input maps."""
    in_maps = []
    for c in range(N_CORES):
        b, g = c // 2, c % 2
        hs = slice(g * HPC, (g + 1) * HPC)
        # [H, emb, d] -> [emb, H*d] for this head group
        wq_g = np.transpose(wq[hs], (1, 0, 2)).reshape(EMB, JC)
        wk_g = np.transpose(wk[hs], (1, 0, 2)).reshape(EMB, JC)
        wv_g = np.transpose(wv[hs], (1, 0, 2)).reshape(EMB, JC)
        in_maps.append({
            "xq": np.ascontiguousarray(_bf16(q[b]).T),
            "xkv": np.ascontiguousarray(_bf16(k_and_v[b]).T),
            "wq": np.ascontiguousarray(_bf16(wq_g)),
            "wk": np.ascontiguousarray(_bf16(wk_g)),
            "wv": np.ascontiguousarray(_bf16(wv_g)),
            "bq": np.ascontiguousarray(np.asarray(bq, np.float32)[hs]
                                       .reshape(JC)),
            "bk": np.ascontiguousarray(np.asarray(bk, np.float32)[hs]
                                       .reshape(JC)),
            "bv": np.ascontiguousarray(np.asarray(bv, np.float32)[hs]
                                       .reshape(JC)),
            "wo": np.ascontiguousarray(
                _bf16(wo)[g * JC:(g + 1) * JC, :]),
        })
    return in_maps


_NC_CACHE = {}


def kernel(q, k_and_v, wq, bq, wk, bk, wv, bv, wo, bo):
    if "nc" not in _NC_CACHE:
        _NC_CACHE["nc"] = build_kernel()
    nc = _NC_CACHE["nc"]
    in_maps = _prep_inputs(q, k_and_v, wq, bq, wk, bk, wv, bv, wo)
    res = run_bass_kernel_spmd(nc, in_maps, core_ids=list(range(N_CORES)))
    bo = np.asarray(bo, np.float32)
    out = np.empty((B, S, EMB), np.float32)
    for b in range(B):
        out[b] = res.results[2 * b]["out"] + res.results[2 * b + 1]["out"] + bo
    return out



# revision 4
# speedup vs baseline: 1.1288x; 1.1288x over previous
"""Multi-head self-attention TRN2 Bass kernel, v7.

Problem: B=4, S=2048, EMB=1024, H=16 heads, dqk=dv=64.
Sharding: 8 cores; core c handles batch b=c//2 and head group g=c%2
(8 heads each). Each core computes its partial output projection;
host sums the two partials per batch and adds bo.

v6 design (vs baseline):
  - zero-padded K: kzp[h] is [128, S] with the head's K^T in its own
    64 partition rows and zeros in the other 64, so score matmuls are
    full-array [128,128]^T x [128,512] ops (the 64-row PE-tile mode
    measures ~1.75x slower per column on HW).
  - quarter units: attention runs in (s-quarter 512) units; both
    heads of a pair share one [128,1024] scores psum tile (par0|par1)
    -> one exp instr per t, and only 2 AV accumulator banks, leaving
    6 banks = 3-deep scores rotation (HW semaphore hops are ~1us, so
    WAR slack must span multiple t-iterations).
  - softmax denominators: reciprocal (bf16) then PE outer-product
    ones[1,64]^T @ recip[1,512] broadcast into an sp psum slot -- no
    DRAM round-trip, no partition_broadcast (broken on HW).
  - fills: V-proj chunks, K/Q projections of later pairs/quarters and
    the output projection of finished quarters are injected into the
    attention t-loops with deadline tracking, so PE never drains.
  - DMA issue is the dominant serial resource on HW (~2-4us of SWDGE
    descriptor generation + dispatch per dma_start, all on the issuing
    sequencer): inputs are consolidated into a handful of big
    rearranged transfers, z-norm shift DMAs issue from the DVE queue
    and output DMAs from the Pool queue, so no single sequencer
    serializes the kernel.
"""

import ml_dtypes
import numpy as np

import concourse.bass as bass
import concourse.tile as tile
from concourse import bacc, mybir
from concourse.bass_utils import run_bass_kernel_spmd

B, S, EMB, H, DH = 4, 2048, 1024, 16, 64
N_CORES = 8
HPC = H // 2          # heads per core
JC = HPC * DH         # 512: per-core projected width
VB = DH + 2           # 66: per-head V block (64 V cols + ones + pad)

F32 = mybir.dt.float32
BF16 = mybir.dt.bfloat16


def build_kernel(reps=1, mode='full'):
    nc = bacc.Bacc(
        "TRN2", target_bir_lowering=False, debug=False, num_devices=N_CORES
    )

    xq = nc.dram_tensor("xq", [EMB, S], BF16, kind="ExternalInput").ap()
    xkv = nc.dram_tensor("xkv", [EMB, S], BF16, kind="ExternalInput").ap()
    wq_d = nc.dram_tensor("wq", [EMB, JC], BF16, kind="ExternalInput").ap()
    wk_d = nc.dram_tensor("wk", [EMB, JC], BF16, kind="ExternalInput").ap()
    wv_d = nc.dram_tensor("wv", [EMB, JC], BF16, kind="ExternalInput").ap()
    bq_d = nc.dram_tensor("bq", [JC], F32, kind="ExternalInput").ap()
    bk_d = nc.dram_tensor("bk", [JC], F32, kind="ExternalInput").ap()
    bv_d = nc.dram_tensor("bv", [JC], F32, kind="ExternalInput").ap()
    wo_d = nc.dram_tensor("wo", [JC, EMB], BF16, kind="ExternalInput").ap()
    out_d = nc.dram_tensor("out", [S, EMB], F32, kind="ExternalOutput").ap()

    import contextlib

    with tile.TileContext(nc) as tc:
        with (
            tc.For_i(0, reps, 1) if reps > 1 else contextlib.nullcontext(),
            tc.tile_pool(name="persist", bufs=1) as pp,
        ):
            qht = [pp.tile([128, S], BF16, name=f"qht{i}") for i in range(4)]
            # zero-padded per-head K^T: head h in rows (h%2)*64..+64
            kzp = [pp.tile([128, S], BF16, name=f"kzp{h}") for h in range(8)]
            vh = [pp.tile([128, HPC * VB], BF16, name=f"vh{t}")
                  for t in range(16)]
            znorm = [pp.tile([128, S], BF16, name=f"zn{i}") for i in range(4)]
            bias_q = pp.tile([128, 4], F32, name="bias_q")
            bias_k = pp.tile([128, 4], F32, name="bias_k")
            bias_v = pp.tile([64, HPC], F32, name="bias_v")  # [d, head]
            ones_t = pp.tile([66, 64], BF16, name="ones_t")

            nc.sync.dma_start(bias_q[:], bq_d.rearrange("(c p) -> p c", p=128))
            nc.sync.dma_start(bias_k[:], bk_d.rearrange("(c p) -> p c", p=128))
            nc.sync.dma_start(bias_v[:], bv_d.rearrange("(h d) -> d h", d=DH))
            nc.vector.memset(ones_t[:], 1.0)
            # zero the dead rows of each kzp tile
            for h in range(8):
                off = 0 if h % 2 == 1 else 64
                nc.vector.memset(kzp[h][off:off + 64, :], 0.0)
            # ones columns in vh blocks (col 64 of each 66-block); pad col 0
            for t in range(16):
                blocks = vh[t][:].rearrange("p (h c) -> p h c", c=VB)
                nc.vector.memset(blocks[:, :, DH:DH + 1], 1.0)
                nc.vector.memset(blocks[:, :, DH + 1:], 0.0)

            with (
                tc.tile_pool(name="sps", bufs=1, space="PSUM") as sp_pool,
                tc.tile_pool(name="avps", bufs=1, space="PSUM") as av_pool,
                tc.tile_pool(name="pt", bufs=5) as pt_pool,
                tc.tile_pool(name="dn", bufs=2) as dn_pool,
                tc.tile_pool(name="dre", bufs=1) as dre_pool,
                tc.tile_pool(name="znsc", bufs=2) as zns_pool,
                tc.tile_pool(name="ostg", bufs=2) as ostg_pool,
                tc.tile_pool(name="xr_p", bufs=1) as xr_pool,
                tc.tile_pool(name="wqkv", bufs=1) as w_pool,
            ):
                # consolidated tiles: e-chunks side by side per partition
                wk_all = w_pool.tile([128, 8 * JC], BF16, name="wk_all")
                wq_all = w_pool.tile([128, 8 * JC], BF16, name="wq_all")
                wv_all = w_pool.tile([128, 8 * JC], BF16, name="wv_all")
                xk_all = xr_pool.tile([128, 8 * S], BF16, name="xk_all")
                xq_all = xr_pool.tile([128, 8 * S], BF16, name="xq_all")
                wks = [wk_all[:, e * JC:(e + 1) * JC] for e in range(8)]
                wqs = [wq_all[:, e * JC:(e + 1) * JC] for e in range(8)]
                wvs = [wv_all[:, e * JC:(e + 1) * JC] for e in range(8)]
                xkr = [xk_all[:, e * S:(e + 1) * S] for e in range(8)]
                xqr = [xq_all[:, e * S:(e + 1) * S] for e in range(8)]
                # big rearranged DMAs; halves keep SWDGE desc count <= 512
                # DRAM (e p) r -> SBUF partition p, col (e r)
                wk_r = wk_d.rearrange("(e p) r -> p e r", p=128)
                wq_r = wq_d.rearrange("(e p) r -> p e r", p=128)
                wv_r = wv_d.rearrange("(e p) r -> p e r", p=128)
                xk_r = xkv.rearrange("(e p) r -> p e r", p=128)
                xq_r = xq.rearrange("(e p) r -> p e r", p=128)

                def big(dst_flat, src_r, w, piece, npiece):
                    dst = dst_flat.rearrange("p (e r) -> p e r", r=w)
                    lo = piece * w // npiece
                    hi = (piece + 1) * w // npiece
                    nc.sync.dma_start(dst[:, :, lo:hi], src_r[:, :, lo:hi])

                big(wk_all[:], wk_r, JC, 0, 2)
                big(wk_all[:], wk_r, JC, 1, 2)
                big(xk_all[:], xk_r, S, 0, 4)
                big(wq_all[:], wq_r, JC, 0, 2)
                big(wq_all[:], wq_r, JC, 1, 2)
                big(xq_all[:], xq_r, S, 0, 4)
                big(wv_all[:], wv_r, JC, 0, 2)
                big(wv_all[:], wv_r, JC, 1, 2)
                for piece in range(1, 4):
                    big(xk_all[:], xk_r, S, piece, 4)
                for piece in range(1, 4):
                    big(xq_all[:], xq_r, S, piece, 4)
                wo_r = wo_d.rearrange("(j p) r -> p j r", p=128)
                wos_all = pp.tile([128, 4 * EMB], BF16, name="wos_all")
                nc.sync.dma_start(
                    wos_all[:].rearrange("p (j r) -> p j r", r=EMB), wo_r)
                wos = [wos_all[:, j * EMB:(j + 1) * EMB] for j in range(4)]

                def sp_tile():
                    # uniform psum tile [128, 1024] f32 (2 banks), 3 tags
                    sp_tile.rot += 1
                    r = sp_tile.rot % 3
                    return sp_pool.tile([128, 1024], F32, tag=f"sp{r}",
                                        name=f"sp{r}")
                sp_tile.rot = 0

                skip_mm = (mode == "nomm")

                def emit_kproj_sc(pair, sc):
                    """K proj column block -> both heads' kzp rows."""
                    jsl = slice(pair * 128, (pair + 1) * 128)
                    ssl = slice(sc * 512, (sc + 1) * 512)
                    ps = sp_tile()
                    for e in ([0] if mode == "projlite" else range(8)) \
                            if not skip_mm else []:
                        nc.tensor.matmul(
                            ps[:, 0:512], wks[e][:, jsl], xkr[e][:, ssl],
                            start=(e == 0), stop=(e == 7 or mode == "projlite"),
                        )
                    for par in range(2):
                        off = par * 64
                        nc.vector.tensor_scalar_add(
                            kzp[pair * 2 + par][off:off + 64, ssl],
                            ps[off:off + 64, 0:512],
                            bias_k[off:off + 64, pair:pair + 1])

                def emit_qproj_sc(pair, sc):
                    jsl = slice(pair * 128, (pair + 1) * 128)
                    ssl = slice(sc * 512, (sc + 1) * 512)
                    ps = sp_tile()
                    for e in ([0] if mode == "projlite" else range(8)) \
                            if not skip_mm else []:
                        nc.tensor.matmul(
                            ps[:, 0:512], wqs[e][:, jsl], xqr[e][:, ssl],
                            start=(e == 0), stop=(e == 7 or mode == "projlite"),
                        )
                    nc.vector.tensor_scalar_add(
                        qht[pair][:, ssl], ps[:, 0:512],
                        bias_q[:, pair:pair + 1])

                def emit_vproj(tch):
                    tsl = slice(tch * 128, (tch + 1) * 128)
                    ps = sp_tile()
                    for e in ([0] if mode == "projlite" else range(8)) \
                            if not skip_mm else []:
                        nc.tensor.matmul(
                            ps[:, 0:512], xkr[e][:, tsl], wvs[e],
                            start=(e == 0), stop=(e == 7 or mode == "projlite"),
                        )
                    nc.vector.tensor_copy(
                        vh[tch][:].rearrange(
                            "p (h c) -> p h c", c=VB)[:, :, 0:DH],
                        ps[:, 0:512].rearrange("p (h d) -> p h d", d=DH),
                    )

                def emit_outproj(scc, oc):
                    psl = slice(scc * 128, (scc + 1) * 128)
                    osl = slice(oc * 512, (oc + 1) * 512)
                    ps = sp_tile()
                    for jt in range(4) if not skip_mm else []:
                        nc.tensor.matmul(
                            ps[:, 0:512], znorm[jt][:, psl], wos[jt][:, osl],
                            start=(jt == 0), stop=(jt == 3),
                        )
                    ostg = ostg_pool.tile([128, 512], F32, tag="ostg",
                                          name="ostg")
                    nc.vector.tensor_copy(ostg[:], ps[:, 0:512])
                    if mode != 'nooutdma':
                        nc.sync.dma_start(out_d[psl, osl], ostg[:])

                # ---- fill queue: (emit_fn, deadline (sq, pair, t)) ----
                fills = []

                def flush_fills(upto):
                    while fills and fills[0][1] <= upto:
                        fills.pop(0)[0]()

                def take_fills(n, now):
                    for _ in range(n):
                        if not fills or fills[0][2] > now:
                            return
                        fills.pop(0)[0]()

                def emit_attn(pair, sq, fill_budget=0):
                    budget = [fill_budget]
                    s0 = sq * 512
                    avs = {}
                    for par in range(2):
                        avs[par] = av_pool.tile(
                            [VB, 512], F32, tag=f"av{par}", name=f"av{par}")

                    def emit_scores(t):
                        sp = sp_tile()
                        sw = 64 if mode == "scoreslite" else 512
                        for par in range(2) if not skip_mm else []:
                            nc.tensor.matmul(
                                sp[:, par * 512:par * 512 + sw],
                                kzp[pair * 2 + par][:,
                                                    t * 128:(t + 1) * 128],
                                qht[pair][:, s0:s0 + sw],
                                start=True, stop=True,
                            )
                        ptt = pt_pool.tile([128, 1024], BF16, tag="ptt",
                                           name="ptt")
                        w = 64 if mode == "smallexp" else 1024
                        nc.scalar.activation(
                            ptt[:, 0:w], sp[:, 0:w],
                            mybir.ActivationFunctionType.Exp, scale=0.125)
                        return ptt

                    def emit_av(t, ptt):
                        aw = 64 if mode == "avlite" else 512
                        for par in range(2) if not skip_mm else []:
                            h = pair * 2 + par
                            nc.tensor.matmul(
                                avs[par][:, 0:aw],
                                vh[t][:, h * VB:(h + 1) * VB],
                                ptt[:, par * 512:par * 512 + aw],
                                start=(t == 0), stop=(t == 15),
                                skip_group_check=True,
                            )

                    prev = None
                    for t in range(16):
                        ptt = emit_scores(t)
                        if prev is not None:
                            emit_av(prev[0], prev[1])
                            n = min(2 if t % 2 == 0 else 1, budget[0])
                            if n:
                                take_fills(n, (sq, pair, t))
                                budget[0] -= n
                        prev = (t, ptt)
                    emit_av(prev[0], prev[1])

                    # ---- normalize ----
                    for par in range(2):
                        h = pair * 2 + par
                        dnb = dn_pool.tile([66, 512], BF16,
                                           tag=f"dn{par}", name=f"dn{par}")
                        with nc.allow_low_precision(reason="1/D bf16 bcast"):
                            nc.vector.reciprocal(
                                dnb[64:65, :], avs[par][DH:DH + 1, :])
                        dps = sp_tile()
                        nc.tensor.matmul(
                            dps[0:64, 0:512], ones_t[64:65, 0:64],
                            dnb[64:65, :], start=True, stop=True)
                        dreb = dre_pool.tile([64, 512], BF16,
                                             tag=f"dreb{par}",
                                             name=f"dreb{par}")
                        nc.vector.tensor_copy(dreb[:], dps[0:64, 0:512])
                        if par == 0:
                            nc.vector.tensor_mul(
                                znorm[pair][0:64, s0:s0 + 512],
                                avs[par][0:DH, :], dreb[:])
                            nc.vector.tensor_scalar_add(
                                znorm[pair][0:64, s0:s0 + 512],
                                znorm[pair][0:64, s0:s0 + 512],
                                bias_v[:, h:h + 1])
                        else:
                            zn_s = zns_pool.tile([64, 512], BF16,
                                                 tag="zn_s", name="zn_s")
                            nc.vector.tensor_mul(
                                zn_s[:], avs[par][0:DH, :], dreb[:])
                            nc.vector.tensor_scalar_add(
                                zn_s[:], zn_s[:], bias_v[:, h:h + 1])
                            nc.sync.dma_start(
                                znorm[pair][64:128, s0:s0 + 512], zn_s[:])

                # ---------------- schedule ----------------
                emit_kproj_sc(0, 0)
                emit_qproj_sc(0, 0)
                emit_vproj(0)
                emit_vproj(1)

                # fill units in deadline order
                RDY0 = (-1, -1, -1)
                fl = []
                for t in range(2, 16):
                    fl.append(((0, 0, t), lambda t=t: emit_vproj(t), RDY0))
                for sc in range(1, 4):
                    fl.append(((0, 0, 4 * sc - 2),
                               lambda sc=sc: emit_kproj_sc(0, sc), RDY0))
                for p in range(1, 4):
                    for sc in range(4):
                        fl.append(((0, p, -1),
                                   lambda p=p, sc=sc: emit_kproj_sc(p, sc),
                                   RDY0))
                    fl.append(((0, p, -1),
                               lambda p=p: emit_qproj_sc(p, 0), RDY0))
                for sq in range(1, 4):
                    for p in range(4):
                        fl.append(((sq, p, -1),
                                   lambda p=p, sq=sq: emit_qproj_sc(p, sq),
                                   RDY0))
                for sq in range(3):
                    for scc in range(4 * sq, 4 * sq + 4):
                        for oc in range(2):
                            # ready only once sweep sq's last unit is done
                            fl.append(((sq + 1, 0, 0),
                                       lambda scc=scc, oc=oc:
                                       emit_outproj(scc, oc),
                                       (sq, 3, 999)))
                fl.sort(key=lambda x: x[0])
                fills.extend((fn, dl, rd) for dl, fn, rd in fl)

                budgets = {}
                for sq in range(4):
                    for p in range(4):
                        budgets[(sq, p)] = 4
                budgets[(0, 0)] = 21
                budgets[(0, 1)] = 5
                budgets[(0, 2)] = 5
                budgets[(0, 3)] = 3

                for sq in range(4):
                    for pair in range(4):
                        flush_fills((sq, pair, -1))
                        emit_attn(pair, sq, budgets[(sq, pair)])
                flush_fills((4, 0, -1))
                for scc in range(12, 16):
                    for oc in range(2):
                        emit_outproj(scc, oc)

    nc.compile()
    return nc


def _bf16(a):
    return np.asarray(a, np.float32).astype(ml_dtypes.bfloat16)


def _prep_inputs(q, k_and_v, wq, bq, wk, bk, wv, bv, wo):
    """Build per-core input maps."""
    in_maps = []
    for c in range(N_CORES):
        b, g = c // 2, c % 2
        hs = slice(g * HPC, (g + 1) * HPC)
        wq_g = np.transpose(wq[hs], (1, 0, 2)).reshape(EMB, JC)
        wk_g = np.transpose(wk[hs], (1, 0, 2)).reshape(EMB, JC)
        wv_g = np.transpose(wv[hs], (1, 0, 2)).reshape(EMB, JC)
        in_maps.append({
            "xq": np.ascontiguousarray(_bf16(q[b]).T),
            "xkv": np.ascontiguousarray(_bf16(k_and_v[b]).T),
            "wq": np.ascontiguousarray(_bf16(wq_g)),
            "wk": np.ascontiguousarray(_bf16(wk_g)),
            "wv": np.ascontiguousarray(_bf16(wv_g)),
            "bq": np.ascontiguousarray(np.asarray(bq, np.float32)[hs]
                                       .reshape(JC)),
            "bk": np.ascontiguousarray(np.asarray(bk, np.float32)[hs]
                                       .reshape(JC)),
            "bv": np.ascontiguousarray(np.asarray(bv, np.float32)[hs]
                                       .reshape(JC)),
            "wo": np.ascontiguousarray(
                _bf16(wo)[g * JC:(g + 1) * JC, :]),
        })
    return in_maps


_NC_CACHE = {}


def kernel(q, k_and_v, wq, bq, wk, bk, wv, bv, wo, bo):
    if "nc" not in _NC_CACHE:
        _NC_CACHE["nc"] = build_kernel()
    nc = _NC_CACHE["nc"]
    in_maps = _prep_inputs(q, k_and_v, wq, bq, wk, bk, wv, bv, wo)
    res = run_bass_kernel_spmd(nc, in_maps, core_ids=list(range(N_CORES)))
    bo = np.asarray(bo, np.float32)
    out = np.empty((B, S, EMB), np.float32)
    for b in range(B):
        out[b] = res.results[2 * b]["out"] + res.results[2 * b + 1]["out"] + bo
    return out
